# revision 1
# baseline (speedup 1.0000x reference)
"""Trainium2 Bass kernel: 3x3 same-padding conv2d, NCHW.

Full inputs: x (32, 64, 112, 112) f32, W (64, 128, 3, 3) f32 (IOHW).
Full output: (32, 128, 112, 112) f32.

Strategy: data-parallel over batch across 8 NeuronCores (4 images/core).
Per core, images are processed as 2 pairs: partitions 0-63 hold the even
image's 64 input channels, partitions 64-127 the odd image's. The 3x3 conv
is 9 shift-offset matmuls (contraction over cin=64) accumulated in PSUM.
The two images run as independent K=64 matmul streams in PE row-groups
{0,1} and {2,3} (tile_position auto-derived from base partition), which the
PE executes concurrently, recovering full-array throughput. fp32r operands
give 1 cycle/row matmul speed at N=448.

Host side pre-pads images to 114x114 (zero border = conv padding) so every
DMA is fully contiguous and no on-chip memsets or edge fixups are needed.
"""

import numpy as np

import concourse.bacc as bacc
import concourse.tile as tile
import concourse.mybir as mybir
from concourse.bass_utils import run_bass_kernel_spmd

F32 = mybir.dt.float32
F32R = mybir.dt.float32r

NCORES = 8
H = W_ = 112
HP = H + 2  # padded
NTAP = 9
NTILE = H // 4  # 28 output-row tiles of 4 rows x 112 cols = 448
TAPS = [(u, v) for u in range(3) for v in range(3)]

_NC_CACHE = []


def _build(repeat=1):
    nc = bacc.Bacc()
    xp_ext = nc.declare_dram_parameter("xp", [2, 128, HP, HP], F32R, isOutput=False)
    wt_ext = nc.declare_dram_parameter("wt", [128, NTAP * 128], F32R, isOutput=False)
    out_ext = nc.declare_dram_parameter("out", [4, 128, H, W_], F32, isOutput=True)

    with tile.TileContext(nc) as tc:
        with (
            tc.tile_pool(name="xpool", bufs=2) as xpool,
            tc.tile_pool(name="wpool", bufs=1) as wpool,
            tc.tile_pool(name="opool", bufs=3) as opool,
            tc.tile_pool(name="psum", bufs=3, space="PSUM") as psum,
        ):
            wt = wpool.tile([128, NTAP * 128], F32R)
            nc.sync.dma_start(out=wt[:], in_=wt_ext[:])

            for pair in [p for _ in range(repeat) for p in range(2)]:
                xt = xpool.tile([128, HP, HP], F32R)
                # progressive chunks: small head chunk lets PE start early;
                # later chunks stream in behind the first tiles' matmuls
                bounds = [0, 10, 36, 62, 88, HP]
                for r_s, r_e in zip(bounds[:-1], bounds[1:]):
                    nc.sync.dma_start(
                        out=xt[:, r_s:r_e, :], in_=xp_ext[pair, :, r_s:r_e, :]
                    )

                for ti in range(NTILE):
                    r0 = 4 * ti
                    ps_l = psum.tile([128, 448], F32)
                    ps_h = psum.tile([128, 448], F32)
                    for t, (u, v) in enumerate(TAPS):
                        nc.tensor.matmul(
                            ps_l[:],
                            wt[0:64, t * 128 : (t + 1) * 128],
                            xt[0:64, r0 + u : r0 + u + 4, v : v + W_],
                            start=(t == 0),
                            stop=(t == NTAP - 1),
                        )
                        nc.tensor.matmul(
                            ps_h[:],
                            wt[64:128, t * 128 : (t + 1) * 128],
                            xt[64:128, r0 + u : r0 + u + 4, v : v + W_],
                            start=(t == 0),
                            stop=(t == NTAP - 1),
                        )
                    o_l = opool.tile([128, 448], F32)
                    o_h = opool.tile([128, 448], F32)
                    nc.vector.tensor_copy(o_l[:], ps_l[:])
                    nc.vector.tensor_copy(o_h[:], ps_h[:])
                    nc.sync.dma_start(
                        out=out_ext[2 * pair, :, r0 : r0 + 4, :], in_=o_l[:]
                    )
                    nc.sync.dma_start(
                        out=out_ext[2 * pair + 1, :, r0 : r0 + 4, :], in_=o_h[:]
                    )
    nc.finalize()
    return nc


def get_nc():
    if not _NC_CACHE:
        _NC_CACHE.append(_build())
    return _NC_CACHE[0]


def make_in_maps(x, W):
    x = np.ascontiguousarray(np.asarray(x, dtype=np.float32))
    W = np.ascontiguousarray(np.asarray(W, dtype=np.float32))
    # lhsT per tap t=(u,v): [cin, cout] = W[:, :, u, v]; layout (cin, tap, cout)
    wt_half = np.ascontiguousarray(W.transpose(0, 2, 3, 1)).reshape(64, NTAP * 128)
    wt = np.concatenate([wt_half, wt_half], axis=0)  # duplicate for both halves
    in_maps = []
    for c in range(NCORES):
        xs = x[c * 4 : (c + 1) * 4].reshape(2, 128, H, W_)
        xp = np.zeros((2, 128, HP, HP), dtype=np.float32)
        xp[:, :, 1 : H + 1, 1 : W_ + 1] = xs
        in_maps.append({"xp": xp, "wt": wt})
    return in_maps


def kernel(x, W):
    nc = get_nc()
    in_maps = make_in_maps(x, W)
    res = run_bass_kernel_spmd(nc, in_maps, list(range(NCORES)))
    out = np.concatenate([res.results[c]["out"] for c in range(NCORES)], axis=0)
    return out



# revision 13
# speedup vs baseline: 1.9229x; 1.9229x over previous
"""Trainium2 Bass kernel: 3x3 same-padding conv2d, NCHW.

Full inputs: x (32, 64, 112, 112) f32, W (64, 128, 3, 3) f32 (IOHW).
Full output: (32, 128, 112, 112) f32.

Strategy: data-parallel over batch across 8 NeuronCores (4 images/core).
The PE cost of a matmul is proportional to the output free size N only
(independent of K), so the kernel packs the cin=64 x 9-tap contraction
into 5 matmuls per 4-row output tile (the K<=128 floor: ceil(576/128)):

  tileA [128, 113, 114] f16: partitions 0-63 = padded image rows 0..112,
        partitions 64-127 = the same image shifted down one row (1..113).
  tileB [128, 112, 114] f16: partitions 0-63 = rows 2..113, partitions
        64-127 = rows 2..113 shifted one column. Built on-chip by DVE
        from tileA (partition-aligned copies), so it costs no HBM traffic.

  t0-t2: K=128 pairs (u=0,v)+(u=1,v) from tileA
  t3:    K=64  single (u=2,v=2) from tileA's shifted half
  t4:    K=128 pair (u=2,v=0)+(u=2,v=1) from tileB

Inputs stream as f16 and outputs DMA out as f16 (PSUM accumulates f32;
total quantization error ~6e-4 of absmax vs the 2e-2 gate), halving
both directions of HBM traffic. PSUM->SBUF conversion copies run on the
Activation engine; output DMAs issue from Activation, input DMAs from SP,
so the two streams interleave on the DMA engines.
"""

import numpy as np

import concourse.bacc as bacc
import concourse.tile as tile
import concourse.mybir as mybir
from concourse.bass_utils import run_bass_kernel_spmd

F32 = mybir.dt.float32
F16 = mybir.dt.float16

NCORES = 8
H = W_ = 112
HP = H + 1  # 113 rows per shifted copy
WP = W_ + 2  # 114 padded cols
NCHUNK = 5
NTILE = H // 4  # 28 output-row tiles of 4 rows x 112 cols = 448

# input row chunks: small head so the PE starts early
BOUNDS = [0, 5, 12, 22, 36, 54, 74, 94, HP]

_NC_CACHE = []


def _build(repeat=1):
    nc = bacc.Bacc()
    xp_ext = nc.declare_dram_parameter("xp", [4, 128, HP, WP], F16, isOutput=False)
    wt_ext = nc.declare_dram_parameter("wt", [128, NCHUNK * 128], F16, isOutput=False)
    out_ext = nc.declare_dram_parameter("out", [4, 128, H, W_], F16, isOutput=True)

    with tile.TileContext(nc) as tc:
        with (
            tc.tile_pool(name="xpool", bufs=2) as xpool,
            tc.tile_pool(name="bpool", bufs=2) as bpool,
            tc.tile_pool(name="wpool", bufs=1) as wpool,
            tc.tile_pool(name="opool", bufs=8) as opool,
            tc.tile_pool(name="psum", bufs=8, space="PSUM") as psum,
        ):
            wt = wpool.tile([128, NCHUNK * 128], F16)
            nc.sync.dma_start(out=wt[:], in_=wt_ext[:])

            for img in [i for _ in range(repeat) for i in range(4)]:
                xt = xpool.tile([128, HP, WP], F16)
                xb = bpool.tile([128, H, WP], F16)
                for ci, (r_s, r_e) in enumerate(zip(BOUNDS[:-1], BOUNDS[1:])):
                    # head chunk of image 0 goes via Pool's SWDGE so its
                    # descriptor gen overlaps the weight DMA's HWDGE gen
                    in_eng = nc.gpsimd if (img == 0 and ci == 0) else nc.sync
                    in_eng.dma_start(
                        out=xt[:, r_s:r_e, :], in_=xp_ext[img, :, r_s:r_e, :]
                    )
                    # xb[p<64, r, c] = xt[p<64, r+2, c]   (= xpad rows 2..)
                    # xb[p>=64, r, c] = xt[p>=64, r+1, c+1] (rows 2.., cols 1..)
                    lo_s, lo_e = max(0, r_s - 2), max(0, r_e - 2)
                    if lo_e > lo_s:
                        nc.vector.tensor_copy(
                            xb[0:64, lo_s:lo_e, :], xt[0:64, lo_s + 2 : lo_e + 2, :]
                        )
                    hi_s, hi_e = max(0, r_s - 1), min(H, r_e - 1)
                    if hi_e > hi_s:
                        nc.vector.tensor_copy(
                            xb[64:128, hi_s:hi_e, 0:113],
                            xt[64:128, hi_s + 1 : hi_e + 1, 1:114],
                        )
                # bottom pad row of the lo half (xpad row 113 = zeros)
                nc.vector.memset(xb[0:64, 111:112, :], 0.0)

                for ti in range(NTILE):
                    r0 = 4 * ti
                    ps = psum.tile([128, 448], F32)
                    for t in range(3):
                        nc.tensor.matmul(
                            ps[:],
                            wt[:, t * 128 : (t + 1) * 128],
                            xt[:, r0 : r0 + 4, t : t + W_],
                            start=(t == 0),
                            stop=False,
                        )
                    # (u=2, v=2) single on tileA's shifted half
                    nc.tensor.matmul(
                        ps[:],
                        wt[64:128, 3 * 128 : 4 * 128],
                        xt[64:128, r0 + 1 : r0 + 5, 2 : 2 + W_],
                        start=False,
                        stop=False,
                    )
                    # (u=2, v=0)+(u=2, v=1) pair on tileB
                    nc.tensor.matmul(
                        ps[:],
                        wt[:, 4 * 128 : 5 * 128],
                        xb[:, r0 : r0 + 4, 0:W_],
                        start=False,
                        stop=True,
                    )
                    # two psum tiles accumulate into one 8-row slab; Pool's
                    # SWDGE issues the slab DMA so Act's SEQ only runs copies
                    if ti % 2 == 0:
                        ot = opool.tile([128, 896], F16)
                    nc.scalar.copy(ot[:, (ti % 2) * 448 : (ti % 2 + 1) * 448], ps[:])
                    if ti % 2 == 1:
                        # final slab of the run: SP/HWDGE has a shorter
                        # latency chain than Pool/SWDGE, shrinking the drain
                        dma_eng = (
                            nc.sync if (img == 3 and ti == NTILE - 1) else nc.gpsimd
                        )
                        dma_eng.dma_start(
                            out=out_ext[img, :, r0 - 4 : r0 + 4, :], in_=ot[:]
                        )
    nc.finalize()
    return nc


def get_nc():
    if not _NC_CACHE:
        _NC_CACHE.append(_build())
    return _NC_CACHE[0]


def make_in_maps(x, W):
    x = np.ascontiguousarray(np.asarray(x, dtype=np.float32))
    W = np.ascontiguousarray(np.asarray(W, dtype=np.float32))
    # lhsT per chunk: [K, cout]. W is (cin, cout, u, v).
    wt = np.zeros((128, NCHUNK * 128), dtype=np.float16)
    for t in range(3):
        wt[0:64, t * 128 : (t + 1) * 128] = W[:, :, 0, t]
        wt[64:128, t * 128 : (t + 1) * 128] = W[:, :, 1, t]
    wt[64:128, 3 * 128 : 4 * 128] = W[:, :, 2, 2]
    wt[0:64, 4 * 128 : 5 * 128] = W[:, :, 2, 0]
    wt[64:128, 4 * 128 : 5 * 128] = W[:, :, 2, 1]
    in_maps = []
    for c in range(NCORES):
        xs = x[c * 4 : (c + 1) * 4]  # [4, 64, 112, 112]
        xpad = np.zeros((4, 64, H + 2, WP), dtype=np.float16)
        xpad[:, :, 1 : H + 1, 1 : W_ + 1] = xs
        xp = np.empty((4, 128, HP, WP), dtype=np.float16)
        xp[:, 0:64] = xpad[:, :, 0:HP]          # rows 0..112 of padded image
        xp[:, 64:128] = xpad[:, :, 1 : HP + 1]  # rows 1..113 (shift by one)
        in_maps.append({"xp": xp, "wt": wt})
    return in_maps


def kernel(x, W):
    nc = get_nc()
    in_maps = make_in_maps(x, W)
    res = run_bass_kernel_spmd(nc, in_maps, list(range(NCORES)))
    out = np.concatenate(
        [res.results[c]["out"].astype(np.float32) for c in range(NCORES)], axis=0
    )
    return out
